# revision 11
# baseline (speedup 1.0000x reference)
"""Causal self-attention (B=2, S=2048, E=2048, H=16, rope) on 8 TRN2 NeuronCores.

Sharding: tensor-parallel over heads. Each core owns 2 heads (both batches):
w_qkv rows / w_out columns for its heads; every core reads the full x
(replicated, bf16, pre-transposed) and produces a partial [B*S, E] f32
output; the host sums the 8 partials (the "all-reduce").

Per-core kernel (v2: flat software pipeline):
  - xT [E, B*S] bf16 serves as matmul rhs (Q/K projections -> QT/KT arrive
    transposed [D, S], the layout attention wants) and as lhsT (V
    projection, natural [S, D]).
  - scores are computed transposed: scoresT[k,q] = KT^T @ QT, in panels of
    512 q columns. exp runs on ScalarE (softmax scale folded into the
    activation scale); causal masking = per-kb column offsets + one bf16
    0/1 mask multiply on the diagonal block; A@V and the sums matmuls
    accumulate only each k-block's causally-valid column range.
  - softmax sums over k (partition dim) use a ones[128,128] matmul that
    produces the column sums already broadcast across all 128 partitions;
    reciprocal + multiply fold normalization into the y^T PSUM evacuation.
  - attn^T feeds A@V as lhsT directly - no transposes anywhere.
  - rope is applied on DVE during QKV-PSUM evacuation with [D, S] cos /
    signed-sin tables; the half-rotation uses a partition-rolled sin table
    so both multiplies are full-width.
  - EMISSION PIPELINE: attention panel p only needs q/k/v of token blocks
    <= p, so panels are processed ascending and interleaved (via emission
    generators) with the projection of token-block p+1 and the
    out-projection of panel p-1. The PE queue then always has independent
    matmul work between a score matmul and the A@V that waits on its exp,
    so ScalarE latency never stalls the PE.
"""

import math

import numpy as np
import ml_dtypes

import concourse.bass as bass
import concourse.mybir as mybir
import concourse.tile as tile
from concourse import bacc
from concourse.bass_utils import run_bass_kernel_spmd

B, S, E, H, D = 2, 2048, 2048, 16, 128
NCORES = 8
HL = H // NCORES            # heads per core = 2
NTOK = B * S                # 4096
KE = E // 128               # 16 contraction chunks
NB = S // 128               # 16 k/token blocks per batch
NPANEL = S // 512           # 4 q panels per batch
SOFTMAX_SCALE = 1.0 / math.sqrt(D)
BF16 = mybir.dt.bfloat16
F32 = mybir.dt.float32

ROPE_BASE = 10000.0


def _rope_tables():
    inv_freq = 1.0 / (ROPE_BASE ** (np.arange(0, D, 2, dtype=np.float32) / D))
    pos = np.arange(S, dtype=np.float32)
    freqs = np.outer(pos, inv_freq)               # [S, D/2]
    emb = np.concatenate([freqs, freqs], -1)      # [S, D]
    cosT = np.cos(emb).T.astype(np.float32)       # [D, S]
    sinT = np.sin(emb).T.astype(np.float32)
    sinS = sinT.copy()
    sinS[: D // 2] *= -1.0                        # signed: rotate_half sign folded in
    return np.ascontiguousarray(cosT), np.ascontiguousarray(sinS)


def _emit(nc, tc, xT, wqkvT, w_outT, out, cos_d, sin_d, mask_d):
    from contextlib import ExitStack

    ctx = ExitStack()
    with ctx:
        singles = ctx.enter_context(tc.tile_pool(name="singles", bufs=1))
        xpool = ctx.enter_context(tc.tile_pool(name="xcol", bufs=2))
        persist = ctx.enter_context(tc.tile_pool(name="persist", bufs=1))
        ropet = ctx.enter_context(tc.tile_pool(name="ropet", bufs=3))
        attnp = ctx.enter_context(tc.tile_pool(name="attn", bufs=6))
        evacp = ctx.enter_context(tc.tile_pool(name="evac", bufs=2))
        outp = ctx.enter_context(tc.tile_pool(name="outp", bufs=4))
        psum = ctx.enter_context(tc.tile_pool(name="psum", bufs=2, space="PSUM"))

        # ---- constant tiles (DMAs for non-critical ones deferred below) ----
        wq_sb = [singles.tile([128, 3 * HL * D], BF16, tag=f"wq{ke}", name=f"wq{ke}")
                 for ke in range(KE)]
        wo_sb = singles.tile([128, HL, E], BF16, tag="wo")
        cos_sb = singles.tile([128, S], F32, tag="cos")
        sin_sb = singles.tile([128, S], F32, tag="sin")
        mask_sb = singles.tile([128, 128], BF16, tag="mask")
        ones_kk = singles.tile([128, 128], BF16, tag="oneskk")
        nc.vector.memset(ones_kk, 1.0)
        # bulky constants ride the SWDGE queues so the HWDGE queues carry
        # only the latency-critical wq/xc stream; wo is deferred past the
        # tb0 bandwidth crunch (first needed at outproj of panel 0)
        nc.gpsimd.dma_start(out=cos_sb, in_=cos_d)
        nc.gpsimd.dma_start(out=sin_sb, in_=sin_d)
        nc.gpsimd.dma_start(out=mask_sb, in_=mask_d)

        # ---- persistent per-(b,h) tensors ----
        q_sb = [[persist.tile([128, S], BF16, tag=f"q{b}{h}", name=f"q{b}{h}") for h in range(HL)] for b in range(B)]
        k_sb = [[persist.tile([128, S], BF16, tag=f"k{b}{h}", name=f"k{b}{h}") for h in range(HL)] for b in range(B)]
        v_sb = [persist.tile([128, NB, HL * D], BF16, tag=f"v{b}", name=f"v{b}") for b in range(B)]
        y_sb = [[persist.tile([128, S], BF16, tag=f"y{b}{h}", name=f"y{b}{h}") for h in range(HL)] for b in range(B)]

        def proj_gen(b, sb4):
            """Project token block (b, sb4) [512 tokens] -> q,k (roped, [D,S]
            transposed layout) and v. Yields after each matmul."""
            tb = b * (S // 512) + sb4
            soff = sb4 * 512
            xc = []
            for ke in range(KE):
                x1 = xpool.tile([128, 512], BF16, tag=f"xc{ke}", name=f"xc{tb}_{ke}")
                if tb == 0:
                    # interleave weight/x loads so matmul ke starts after
                    # ~2 small DMAs instead of after the whole input load
                    nc.sync.dma_start(
                        out=wq_sb[ke], in_=wqkvT[ke * 128:(ke + 1) * 128, :]
                    )
                nc.sync.dma_start(
                    out=x1,
                    in_=xT[ke * 128:(ke + 1) * 128, tb * 512:(tb + 1) * 512],
                )
                xc.append(x1)
            if tb == 1:
                for hl in range(HL):
                    nc.gpsimd.dma_start(
                        out=wo_sb[:, hl, :], in_=w_outT[hl * 128:(hl + 1) * 128, :]
                    )
            # prefix yield: lets the driver pull in just the DMA triggers
            # of the next token block before the previous segment drains
            yield
            # 8 accumulation chains (4 QK rows + 4 V token-blocks); tb 0 is
            # DMA-paced so advance chains in pairs per-ke, otherwise one
            # chain at a time (16 matmuls back to back, then evacuate)
            chains = [("qk", rb) for rb in range(2 * HL)] + [
                ("v", tsb) for tsb in range(4)
            ]
            if tb == 0:
                waves = [chains[i:i + 2] for i in range(0, 8, 2)]
            else:
                waves = [[c] for c in chains]
            for wv, wave in enumerate(waves):
                pss = [
                    psum.tile([128, 512], F32, tag="ps", bufs=3,
                              name=f"p{tb}_{wv}{j}")
                    for j in range(len(wave))
                ]
                for ke in range(KE):
                    for j, (kind, idx) in enumerate(wave):
                        if kind == "qk":
                            nc.tensor.matmul(
                                pss[j],
                                lhsT=wq_sb[ke][:, idx * 128:(idx + 1) * 128],
                                rhs=xc[ke],
                                start=(ke == 0),
                                stop=(ke == KE - 1),
                            )
                        else:
                            nc.tensor.matmul(
                                pss[j][:, 0:HL * D],
                                lhsT=xc[ke][:, idx * 128:(idx + 1) * 128],
                                rhs=wq_sb[ke][:, 2 * HL * 128:],
                                start=(ke == 0),
                                stop=(ke == KE - 1),
                            )
                        yield
                for j, (kind, idx) in enumerate(wave):
                    ps = pss[j]
                    if kind == "qk":
                        rb = idx
                        # rope: dst = t*cos + swap(t)*sin_signed, bf16 out
                        dst = (q_sb if rb < HL else k_sb)[b][rb % HL]
                        sl = bass.ds(soff, 512)
                        tsw = ropet.tile([128, 512], F32, tag="tsw", name=f"tsw{tb}{rb}")
                        tco = ropet.tile([128, 512], F32, tag="tco", name=f"tco{tb}{rb}")
                        nc.vector.tensor_mul(tsw[0:64, :], ps[64:128, :], sin_sb[0:64, sl])
                        nc.vector.tensor_mul(tsw[64:128, :], ps[0:64, :], sin_sb[64:128, sl])
                        nc.vector.tensor_mul(tco, ps, cos_sb[:, sl])
                        nc.vector.tensor_add(dst[:, sl], tco, tsw)
                    else:
                        blk = (soff // 128) + idx
                        nc.vector.tensor_copy(v_sb[b][:, blk, :], ps[:, 0:HL * D])

        def attn_gen(b, p):
            """Attention for q panel p of batch b, both heads. Software
            pipelined: A@V/sums for block kb are emitted one step after its
            scores/exp, so interleaved filler matmuls (not exp latency)
            occupy the PE in between. Yields once per kb step."""
            nkb = 4 * p + 4
            for hl in range(HL):
                yps = psum.tile([128, 512], F32, tag="yps", bufs=2, name=f"yps{b}{hl}{p}")
                sps = psum.tile([128, 512], F32, tag="sps", bufs=1, name=f"sps{b}{hl}{p}")
                pend = None  # (kb, at, qoff) waiting for its A@V + sums
                for kb in range(nkb):
                    # kb's causally-valid q columns within the panel start at
                    # qoff; kb=0 always has qoff=0 (start=True initializes all
                    # columns), so later kbs may accumulate partial column
                    # ranges - no zero-padding needed
                    qoff = max(0, kb - 4 * p) * 128
                    at = attnp.tile([128, 512], BF16, tag="attn", name=f"at{b}{hl}{p}{kb}")
                    ps = psum.tile([128, 512], F32, tag="sc", bufs=2, name=f"sc{b}{hl}{p}{kb}")
                    nc.tensor.matmul(
                        ps[:, 0:512 - qoff],
                        lhsT=k_sb[b][hl][:, kb * 128:(kb + 1) * 128],
                        rhs=q_sb[b][hl][:, p * 512 + qoff:(p + 1) * 512],
                        start=True,
                        stop=True,
                    )
                    nc.scalar.activation(
                        at[:, qoff:512],
                        ps[:, 0:512 - qoff],
                        mybir.ActivationFunctionType.Exp,
                        scale=SOFTMAX_SCALE,
                    )
                    if kb >= 4 * p:  # diagonal block: zero the k>q half
                        nc.vector.tensor_mul(
                            at[:, qoff:qoff + 128], at[:, qoff:qoff + 128], mask_sb
                        )
                    if pend is not None:
                        _attn_tail(b, hl, p, nkb, yps, sps, *pend)
                    pend = (kb, at, qoff)
                    yield
                _attn_tail(b, hl, p, nkb, yps, sps, *pend)
                rb_sb = evacp.tile([128, 512], F32, tag="rb", name=f"rb{b}{hl}{p}")
                nc.vector.reciprocal_approx_fast(out=rb_sb, in_=sps)
                nc.vector.tensor_mul(y_sb[b][hl][:, p * 512:(p + 1) * 512], yps, rb_sb)
                yield

        def _attn_tail(b, hl, p, nkb, yps, sps, kb, at, qoff):
            nc.tensor.matmul(
                yps[:, qoff:512],
                lhsT=v_sb[b][:, kb, hl * D:(hl + 1) * D],
                rhs=at[:, qoff:512],
                start=(kb == 0),
                stop=(kb == nkb - 1),
            )
            nc.tensor.matmul(
                sps[:, qoff:512],
                lhsT=ones_kk,
                rhs=at[:, qoff:512],
                start=(kb == 0),
                stop=(kb == nkb - 1),
            )

        def outproj_gen(b, p):
            """Out-projection of panel p's tokens. Yields per 2-matmul chain.
            Shares the "ps" PSUM ring with proj chains (never live at once
            in the same segment position). Evacuation rides DVE only (so
            ScalarE stays exp-only and the sc ring is never delayed), and
            the out DMA is triggered from the otherwise-idle GpSimd engine
            so the Sync engine keeps only the latency-critical x stream."""
            for tkb in range(4 * p, 4 * p + 4):
                tok0 = b * S + tkb * 128
                for oc in range(E // 512):
                    ops = psum.tile([128, 512], F32, tag="ps", bufs=3, name=f"o{b}{tkb}{oc}")
                    for hl in range(HL):
                        nc.tensor.matmul(
                            ops,
                            lhsT=y_sb[b][hl][:, tkb * 128:(tkb + 1) * 128],
                            rhs=wo_sb[:, hl, oc * 512:(oc + 1) * 512],
                            start=(hl == 0),
                            stop=(hl == HL - 1),
                        )
                    ot = outp.tile([128, 512], F32, tag="ot", name=f"ot{b}{tkb}{oc}")
                    nc.vector.tensor_copy(ot, ops)
                    nc.gpsimd.dma_start(
                        out=out[tok0:tok0 + 128, oc * 512:(oc + 1) * 512], in_=ot
                    )
                    yield

        def run_seg(attn, n_steps, fillers, n_fill):
            """Interleave one attention generator with filler generators:
            one filler advance up front (emits the next block's DMA
            triggers), then after attention step i advance fillers to
            floor((i+1)*nf/ns)."""
            fill = (x for g in fillers for x in g)
            done = 0
            if attn is not None:
                if next(fill, StopIteration) is not StopIteration:
                    done = 1
                i = 0
                for _ in attn:
                    i += 1
                    want = (i * n_fill) // max(n_steps, 1)
                    while done < want and next(fill, StopIteration) is not StopIteration:
                        done += 1
            for _ in fill:
                pass

        N_PROJ = 8 * KE + 1   # proj yields per token block (incl. DMA prefix)
        N_OUTP = 16           # outproj yields per panel

        # ---- flat pipeline ----
        run_seg(None, 0, [proj_gen(0, 0)], N_PROJ)
        for b in range(B):
            for p in range(NPANEL):
                nkb = 4 * p + 4
                n_steps = HL * (nkb + 1)
                fillers, n_fill = [], 0
                if p < NPANEL - 1:
                    fillers.append(proj_gen(b, p + 1))
                    n_fill += N_PROJ
                elif b == 0:
                    fillers.append(proj_gen(1, 0))
                    n_fill += N_PROJ
                if p > 0:
                    fillers.append(outproj_gen(b, p - 1))
                    n_fill += N_OUTP
                elif b == 1:
                    fillers.append(outproj_gen(0, NPANEL - 1))
                    n_fill += N_OUTP
                run_seg(attn_gen(b, p), n_steps, fillers, n_fill)
        run_seg(None, 0, [outproj_gen(1, NPANEL - 1)], N_OUTP)


def build():
    nc = bacc.Bacc("TRN2", target_bir_lowering=False, debug=False)
    xT = nc.dram_tensor("xT", [E, NTOK], BF16, kind="ExternalInput").ap()
    wqkvT = nc.dram_tensor("wqkvT", [E, 3 * HL * D], BF16, kind="ExternalInput").ap()
    w_outT = nc.dram_tensor("w_outT", [HL * D, E], BF16, kind="ExternalInput").ap()
    out = nc.dram_tensor("out", [NTOK, E], F32, kind="ExternalOutput").ap()

    cosT, sinS = _rope_tables()
    cos_d = nc.inline_tensor(cosT, name="cos_t").ap()
    sin_d = nc.inline_tensor(sinS, name="sin_t").ap()
    # maskT01[k, q] = 1 where k <= q (valid), else 0 — transposed-causal
    mask = np.triu(np.ones((128, 128), np.float32)).astype(ml_dtypes.bfloat16)
    mask_d = nc.inline_tensor(mask, name="maskT01").ap()

    with tile.TileContext(nc) as tc:
        _emit(nc, tc, xT, wqkvT, w_outT, out, cos_d, sin_d, mask_d)
    nc.compile()
    return nc


def make_in_maps(x, w_qkv, w_out):
    bf = ml_dtypes.bfloat16
    x2 = np.asarray(x, np.float32).reshape(NTOK, E)
    xT = np.ascontiguousarray(x2.astype(bf).T)                      # [E, NTOK]
    w_qkv = np.asarray(w_qkv, np.float32)
    w_out = np.asarray(w_out, np.float32)
    in_maps = []
    for c in range(NCORES):
        hs = [HL * c + j for j in range(HL)]
        rows = np.concatenate(
            [w_qkv[t * E + h * D:t * E + (h + 1) * D] for t in range(3) for h in hs]
        )                                                           # [768, E]
        wqkvT = np.ascontiguousarray(rows.astype(bf).T)             # [E, 768]
        w_outT = np.ascontiguousarray(
            w_out[:, c * HL * D:(c + 1) * HL * D].astype(bf).T      # [256, E]
        )
        in_maps.append({"xT": xT, "wqkvT": wqkvT, "w_outT": w_outT})
    return in_maps


_NC = None


def kernel(x, w_qkv, w_out):
    global _NC
    if _NC is None:
        _NC = build()
    in_maps = make_in_maps(x, w_qkv, w_out)
    res = run_bass_kernel_spmd(_NC, in_maps, core_ids=list(range(NCORES)))
    total = np.zeros((NTOK, E), np.float32)
    for r in res.results:
        total += r["out"]
    return total.reshape(B, S, E)
